# revision 24
# baseline (speedup 1.0000x reference)
"""CfC cell (dense MLP) on 8 Trainium2 NeuronCores — data-parallel over batch.

Math (per sample):
    x  = concat(input, hx)                       # [1024]
    x1 = 1.7159*tanh(0.666*(Wb1 x + bb1))        # backbone 1, [1024]
    x2 = 1.7159*tanh(0.666*(Wb2 x1 + bb2))       # backbone 2, [1024]
    ff1 = tanh(W_ff1 x2 + b_ff1); ff2 = tanh(W_ff2 x2 + b_ff2)
    t   = sigmoid((W_ta x2 + b_ta)*ts + W_tb x2 + b_tb)
    out = ff1 + t*(ff2 - ff1)                    # [512]

Device layout: all activations live transposed as [feature_partition,
batch_free]; weights are host-pre-transposed to [K, N] so every GEMM is a
straight lhsT.T @ rhs chain with no on-device transposes. The 1.7159
LeCun gains are folded into the *next* layer's weights, the 0.666 input
scales into the ACT instruction's free affine + pre-scaled biases, so
each layer is exactly matmul-accumulate -> one ACT op.

The gate path (t_a, t_b heads) runs in fp8e4m3 with DoubleRow matmuls
(2x PE throughput): since ts is per-sample, (W_ta x2)*ts = W_ta (x2*ts),
so both gate GEMMs accumulate into ONE PSUM bank
    P = s * (W_ta (x2*ts) + W_tb x2),  s = 2048 (weight fp8 scaling)
and the sigmoid reads P directly with scale ±1/s (sigmoid(-v) = 1-t).
Gate biases are zero for this model; a general bias path is compiled in
only when they are nonzero. fp8 error on the gate path is damped by the
sigmoid slope: measured end-to-end rel err ~1.5e-2 < 2e-2 budget.
The backbone and ff GEMM inputs are fp16 (fp32 PSUM accumulation);
the head epilogue runs in fp16 and the output DMA is fp16.

Batch 8192 is split 1024/core across the 8 cores; weights are replicated.
"""
import os
from contextlib import ExitStack

import numpy as np
import ml_dtypes

IN, HID, BB, B = 512, 512, 1024, 8192
N_CORES = 8
BL = B // N_CORES        # 1024 batch rows per core
K1 = IN + HID            # contraction dim of backbone layer 1 (== BB here)
KT = K1 // 128           # 8 k-tiles (also BB//128)
NB = BL // 512           # 2 batch chunks of 512 (PSUM bank = 512 fp32)
NT1 = BB // 128          # 8 output tiles for backbone layers
NTH = HID // 128         # 4 output tiles per head
LA, LBc = 1.7159, 0.666  # LeCun tanh gain / input scale
GS = 2048.0              # fp8 gate-weight scale (power of 2)

_F16 = np.float16
_F8 = ml_dtypes.float8_e4m3

_cache: dict = {}

# Set by each kernel() call when tracing is enabled (BASS_KERNEL_TRACE=1).
LAST_EXEC_TIME_NS = None


def _install_ntff_shim():
    """Recreate the missing ``antenv.axon_hooks`` so trace=True works."""
    import sys, types, ctypes, contextlib

    if "antenv.axon_hooks" in sys.modules:
        return
    so_path = "/opt/axon/libaxon_pjrt.so"
    try:
        lib = ctypes.CDLL(so_path)
    except OSError:
        return
    if not hasattr(lib, "axon_start_nrt_profile"):
        return
    lib.axon_start_nrt_profile.argtypes = [ctypes.POINTER(ctypes.c_int64), ctypes.c_size_t]
    lib.axon_start_nrt_profile.restype = ctypes.c_int64
    lib.axon_stop_nrt_profile.argtypes = [ctypes.c_char_p]
    lib.axon_stop_nrt_profile.restype = ctypes.c_int64

    @contextlib.contextmanager
    def _hook(output_dir, device_ids):
        import jax

        jax.devices()
        if device_ids:
            ids = (ctypes.c_int64 * len(device_ids))(*device_ids)
            rc = lib.axon_start_nrt_profile(ids, len(device_ids))
        else:
            rc = lib.axon_start_nrt_profile(None, 0)
        if rc != 0:
            raise RuntimeError(f"axon_start_nrt_profile rc={rc}")
        try:
            yield
        finally:
            n = lib.axon_stop_nrt_profile(str(output_dir).encode())
            if n < 0:
                raise RuntimeError(f"axon_stop_nrt_profile rc={n}")

    mod = types.ModuleType("antenv.axon_hooks")
    mod.get_axon_ntff_profile_hook = lambda: _hook

    def set_axon_ntff_profile_hook(h):
        mod.get_axon_ntff_profile_hook = lambda: h

    mod.set_axon_ntff_profile_hook = set_axon_ntff_profile_hook
    sys.modules["antenv.axon_hooks"] = mod
    import antenv

    antenv.axon_hooks = mod


def _build(gate_bias: bool):
    from concourse import bacc, tile, mybir

    f32 = mybir.dt.float32
    f16 = mybir.dt.float16
    f8 = mybir.dt.float8e4
    Tanh = mybir.ActivationFunctionType.Tanh
    Sigm = mybir.ActivationFunctionType.Sigmoid
    DR = mybir.MatmulPerfMode.DoubleRow

    nc = bacc.Bacc("TRN2", target_bir_lowering=False, debug=False, num_devices=N_CORES)

    # xt and w1t packed side by side so one DMA per k-tile delivers a
    # complete (xt_k, w1_k) operand pair; columns are [xt_b0 | w1 | xt_b1].
    # l1h is a fully contiguous copy of k0's [w1 n0 | xt_b0] head piece so
    # the very first matmul's operands ride one dense descriptor chain.
    # w2t/wht/xtb1 use the k-grouped [128, KT, N] layout so each loads with
    # one or two dma_starts (fewer transfers -> shorter end-of-kernel drain).
    l1h = nc.declare_dram_parameter("l1h", [128, 640], f16, isOutput=False)
    l1p = nc.declare_dram_parameter("l1p", [K1, 512 + BB], f16, isOutput=False)
    xtb1 = nc.declare_dram_parameter("xtb1", [128, KT, 512], f16, isOutput=False)
    w2t = nc.declare_dram_parameter("w2t", [128, KT, BB], f16, isOutput=False)
    wht = nc.declare_dram_parameter("wht", [128, KT, 2 * HID], f16, isOutput=False)
    w8g = nc.declare_dram_parameter("w8g", [128, KT, 2 * HID], f8, isOutput=False)
    biases = nc.declare_dram_parameter("biases", [128, 32], f32, isOutput=False)
    tsb = nc.declare_dram_parameter("tsb", [128, BL], f16, isOutput=False)
    out = nc.declare_dram_parameter("out", [HID, BL], f16, isOutput=True)

    # HAM warm-up, emitted BEFORE the TileContext so it lands in the "main"
    # basic block: the PE executes these as soon as its queue comes up
    # (~6us, during the other engines' preamble), so the slow p-state ramp
    # (~5.7us busy time to max clock) burns off before real operands land.
    # Operand values are irrelevant (uninitialized SBUF, dead PSUM bank).
    with nc.sbuf_tensor([128, 256], f16) as warm0, nc.psum_tensor(
        [128, 256], f32
    ) as wacc0:
        N_WARM0 = 14
        for i in range(N_WARM0):
            nc.tensor.matmul(
                wacc0[:], warm0[:, :128], warm0[:],
                start=(i == 0), stop=(i == N_WARM0 - 1),
            )

    with tile.TileContext(nc) as tc, ExitStack() as ctx:
        sb = ctx.enter_context(tc.tile_pool(name="sb", bufs=1))
        tmp = ctx.enter_context(tc.tile_pool(name="tmp", bufs=2))
        ps = ctx.enter_context(tc.tile_pool(name="ps", bufs=8, space="PSUM"))

        # Ring assignment: the sync HWDGE ring carries everything big, in
        # consumption order — l1h first (the first matmul's operands), then
        # the l1p/w2t/w8g/wht input stream, then (issued per head group) the
        # output flush. With no out transfers wedged mid-stream the input
        # prefix drains by ~45us, so output DMAs issued later stream out the
        # moment they are produced. The scalar ring only carries the tiny
        # bias tile, which lands during the preamble window.
        # l1_t keeps only the [xt_b0 | w1] columns; xt_b1 lives in its own
        # k-grouped tile filled by four transfers.
        l1_t = [sb.tile([128, 512 + BB], f16, tag=f"l1{k}", name=f"l1{k}") for k in range(KT)]
        # l1h columns: [w1 n0 (128) | xt_b0 (512)]; split so the very first
        # (half-width) matmul's operands land after only 96KB of stream.
        l1h_t = sb.tile([128, 640], f16, tag="l1h")
        nc.sync.dma_start(l1h_t[:, :384], l1h[:, :384])
        nc.sync.dma_start(l1h_t[:, 384:], l1h[:, 384:])
        nc.sync.dma_start(l1_t[0][:, 640:1024], l1p[0:128, 640:1024])
        nc.sync.dma_start(l1_t[0][:, 1024 : 512 + BB], l1p[0:128, 1024 : 512 + BB])
        nc.sync.dma_start(l1_t[1][:, :1024], l1p[128:256, :1024])
        nc.sync.dma_start(l1_t[1][:, 1024 : 512 + BB], l1p[128:256, 1024 : 512 + BB])
        nc.sync.dma_start(l1_t[2][:, :1024], l1p[256:384, :1024])
        nc.sync.dma_start(l1_t[2][:, 1024 : 512 + BB], l1p[256:384, 1024 : 512 + BB])
        for k in range(3, KT):
            rows = slice(128 * k, 128 * (k + 1))
            nc.sync.dma_start(l1_t[k][:, : 512 + BB], l1p[rows, : 512 + BB])
        xtb1_t = sb.tile([128, KT, 512], f16, tag="xtb1")
        for q in range(4):
            nc.sync.dma_start(
                xtb1_t[:, 2 * q : 2 * q + 2, :], xtb1[:, 2 * q : 2 * q + 2, :]
            )
        bias_t = sb.tile([128, 32], f32, tag="bias")
        nc.sync.dma_start(bias_t[:], biases[:])
        w2_t = sb.tile([128, KT, BB], f16, tag="w2")
        nc.sync.dma_start(w2_t[:, : KT // 2, :], w2t[:, : KT // 2, :])
        nc.sync.dma_start(w2_t[:, KT // 2 :, :], w2t[:, KT // 2 :, :])
        tsb_t = sb.tile([128, BL], f16, tag="tsb")
        nc.sync.dma_start(tsb_t[:], tsb[:])
        w8g_t = sb.tile([128, KT, 2 * HID], f8, tag="w8g")
        nc.sync.dma_start(w8g_t[:], w8g[:])
        wh_t = sb.tile([128, KT, 2 * HID], f16, tag="wh")
        nc.sync.dma_start(wh_t[:], wht[:])

        h1_t = [sb.tile([128, BL], f16, tag=f"h1{n}", name=f"h1{n}") for n in range(NT1)]
        h2_t = [sb.tile([128, BL], f16, tag=f"h2{n}", name=f"h2{n}") for n in range(NT1)]
        # fp8 copies of x2 (and x2*ts) for the DoubleRow gate GEMMs, laid
        # out [128, kgroup, batch] so dim1 pairs feed DoubleRow directly.
        x28_t = sb.tile([128, KT, BL], f8, tag="x28")
        x2s8_t = sb.tile([128, KT, BL], f8, tag="x2s8")

        # backbone layer 1: h1 = tanh(0.666*(W1 x) + 0.666*bb1), fp16 out.
        # k-OUTER so each (xt_k, w1_k) pair is consumed for all 8 n-tiles the
        # moment its DMA lands — the PE streams during the input transfer
        # instead of stalling on the last k-tile. Needs 8 live PSUM banks.
        for b in range(NB):
            bsl = slice(512 * b, 512 * (b + 1))
            accs1 = [
                ps.tile([128, 512], f32, tag="ps", name=f"ps1_{b}_{n}")
                for n in range(NT1)
            ]
            # column layout of l1_t: [xt_b0 (512) | w1 (1024)]; xt_b1 in the
            # k-grouped xtb1 tile; k0's w1-n0 + xt_b0 live in the l1h head
            # tile (columns [w1n0 | xt_b0]). The very first matmul is split
            # into two 256-wide halves so it can start as soon as the first
            # 96KB of the stream lands.
            for k in range(KT):
                if k == 0 and b == 0:
                    rhs = l1h_t[:, 128:640]
                elif b == 0:
                    rhs = l1_t[k][:, 0:512]
                else:
                    rhs = xtb1_t[:, k, :]
                for n in range(NT1):
                    if k == 0 and n == 0:
                        lhsT = l1h_t[:, 0:128]
                    else:
                        lhsT = l1_t[k][:, 512 + 128 * n : 512 + 128 * (n + 1)]
                    nc.tensor.matmul(
                        accs1[n][:],
                        lhsT,
                        rhs,
                        start=(k == 0),
                        stop=(k == KT - 1),
                    )
            for n in range(NT1):
                nc.scalar.activation(
                    h1_t[n][:, bsl], accs1[n][:], Tanh, bias=bias_t[:, n : n + 1], scale=LBc
                )

        # backbone layer 2 (1.7159 folded into w2t on host). Each n-tile's
        # PSUM is evicted to fp16 h2 (scalar ACT); the fp8 gate operands are
        # produced on the vector engine from the fp16 copy.
        for b in range(NB):
            bsl = slice(512 * b, 512 * (b + 1))
            for n in range(NT1):
                acc = ps.tile([128, 512], f32, tag="ps", name=f"ps2_{b}_{n}")
                for k in range(KT):
                    nc.tensor.matmul(
                        acc[:],
                        w2_t[:, k, 128 * n : 128 * (n + 1)],
                        h1_t[k][:, bsl],
                        start=(k == 0),
                        stop=(k == KT - 1),
                    )
                nc.scalar.activation(
                    h2_t[n][:, bsl], acc[:], Tanh, bias=bias_t[:, 8 + n : 9 + n], scale=LBc
                )
                nc.vector.tensor_scalar_add(x28_t[:, n, bsl], h2_t[n][:, bsl], 0.0)
                nc.vector.tensor_mul(x2s8_t[:, n, bsl], h2_t[n][:, bsl], tsb_t[:, bsl])

        if gate_bias:
            # general path: gbias_n = GS*(b_ta*ts + b_tb), added to the gate
            # PSUM before the sigmoids (cols 24..27 = GS*b_ta, 28..31 = GS*b_tb)
            gb_t = [sb.tile([128, BL], f32, tag=f"gb{n}", name=f"gb{n}") for n in range(NTH)]
            for n in range(NTH):
                nc.vector.tensor_scalar(
                    gb_t[n][:], tsb_t[:], bias_t[:, 24 + n : 25 + n],
                    bias_t[:, 28 + n : 29 + n],
                    op0=mybir.AluOpType.mult, op1=mybir.AluOpType.add,
                )

        # heads: the fused gate PSUM (t_a*ts + t_b, fp8 DoubleRow) comes
        # first so the sigmoid chain overlaps the ff GEMMs; only
        # ACT(ff2) -> mul -> add trails the last matmul.
        def head_group(b, n, lo, w, sub, ring):
            """One head pipeline over batch cols [512*b+lo, 512*b+lo+w)."""
            bsl = slice(512 * b + lo, 512 * b + lo + w)
            sfx = f"_{b}_{n}_{sub}"

            gate_ps = ps.tile([128, w], f32, tag="ps", name="ps_g" + sfx)
            for h in range(2):  # 0: ta on x2*ts, 1: tb on x2
                rhs_t = x2s8_t if h == 0 else x28_t
                col = 512 * h + 128 * n
                for i in range(KT // 2):
                    nc.tensor.matmul(
                        gate_ps[:],
                        w8g_t[:, 2 * i : 2 * i + 2, col : col + 128],
                        rhs_t[:, 2 * i : 2 * i + 2, bsl],
                        start=(h == 0 and i == 0),
                        stop=(h == 1 and i == KT // 2 - 1),
                        perf_mode=DR,
                    )
            if gate_bias:
                nc.vector.tensor_add(gate_ps[:], gate_ps[:], gb_t[n][:, bsl])
            tpos = tmp.tile([128, w], f16, tag="tpos", name="tpos" + sfx)
            nc.scalar.activation(tpos[:], gate_ps[:], Sigm, scale=1.0 / GS)
            tneg = tmp.tile([128, w], f16, tag="tneg", name="tneg" + sfx)
            nc.scalar.activation(tneg[:], gate_ps[:], Sigm, scale=-1.0 / GS)

            def ff_mms(h, acc):
                col = 512 * h + 128 * n
                for k in range(KT):
                    nc.tensor.matmul(
                        acc[:],
                        wh_t[:, k, col : col + 128],
                        h2_t[k][:, bsl],
                        start=(k == 0),
                        stop=(k == KT - 1),
                    )

            def bc(h):
                c = 16 + 4 * h + n
                return bias_t[:, c : c + 1]

            ff1_ps = ps.tile([128, w], f32, tag="ps", name="ps_ff1" + sfx)
            ff_mms(0, ff1_ps)
            ff1 = tmp.tile([128, w], f16, tag="ff1", name="ff1" + sfx)
            nc.scalar.activation(ff1[:], ff1_ps[:], Tanh, bias=bc(0))
            u = tmp.tile([128, w], f16, tag="u", name="u" + sfx)
            nc.vector.tensor_mul(u[:], tneg[:], ff1[:])  # (1-t)*ff1

            ff2_ps = ps.tile([128, w], f32, tag="ps", name="ps_ff2" + sfx)
            ff_mms(1, ff2_ps)
            ff2 = tmp.tile([128, w], f16, tag="ff2", name="ff2" + sfx)
            nc.scalar.activation(ff2[:], ff2_ps[:], Tanh, bias=bc(1))
            m2 = tmp.tile([128, w], f16, tag="m2", name="m2" + sfx)
            nc.vector.tensor_mul(m2[:], tpos[:], ff2[:])  # t*ff2
            o = tmp.tile([128, w], f16, tag="o", name="o" + sfx)
            nc.vector.tensor_add(o[:], u[:], m2[:])
            ring.dma_start(out[128 * n : 128 * (n + 1), bsl], o[:])

        for b in range(NB):
            for n in range(NTH):
                if b == NB - 1 and n == NTH - 1:
                    # Final group: two half-batch pipelines, so the first
                    # half's epilogue overlaps the second half's GEMMs and
                    # only a 256-wide ACT->mul->add->DMA trails the last MM.
                    head_group(b, n, 0, 256, 0, nc.sync)
                    head_group(b, n, 256, 256, 1, nc.sync)
                else:
                    # Sync ring: its input prefix has long drained by the
                    # time head groups finish, so outputs stream immediately.
                    head_group(b, n, 0, 512, 0, nc.sync)

    nc.finalize()
    return nc


def _kgroup(a):
    """[K, N] -> [128, KT, N] so [:, k, :] is k-tile k's rows."""
    return np.ascontiguousarray(a.reshape(KT, 128, a.shape[1]).transpose(1, 0, 2))


def _prep_shared(Wb1, bb1, Wb2, bb2, W_ff1, b_ff1, W_ff2, b_ff2, W_ta, b_ta, W_tb, b_tb):
    """Host-side weight layout: transpose to [K, N], fold LeCun gains."""
    w1t = Wb1.T.astype(_F16)
    w2t = _kgroup((LA * Wb2).T).astype(_F16)
    wht = _kgroup(
        np.concatenate([(LA * W).T for W in (W_ff1, W_ff2)], axis=1)
    ).astype(_F16)
    # gate weights: fp8 e4m3, scaled by GS, DoubleRow layout [128, kgroup, M]
    wg = np.concatenate([(GS * LA * W).T for W in (W_ta, W_tb)], axis=1)  # [K, 1024]
    w8g = np.ascontiguousarray(
        wg.reshape(KT, 128, 2 * HID).transpose(1, 0, 2)
    ).astype(_F8)

    biases = np.zeros((128, 32), np.float32)
    biases[:, 0:8] = (LBc * bb1).reshape(8, 128).T
    biases[:, 8:16] = (LBc * bb2).reshape(8, 128).T
    for h, bh in enumerate((b_ff1, b_ff2)):
        biases[:, 16 + 4 * h : 20 + 4 * h] = bh.reshape(4, 128).T
    gate_bias = bool(np.any(b_ta) or np.any(b_tb))
    if gate_bias:
        biases[:, 24:28] = (GS * b_ta).reshape(4, 128).T
        biases[:, 28:32] = (GS * b_tb).reshape(4, 128).T
    return w1t, w2t, wht, w8g, biases, gate_bias


def kernel(input, hx, ts, Wb1, bb1, Wb2, bb2, W_ff1, b_ff1, W_ff2, b_ff2, W_ta, b_ta, W_tb, b_tb):
    global LAST_EXEC_TIME_NS
    from concourse.bass_utils import run_bass_kernel_spmd

    trace = os.environ.get("BASS_KERNEL_TRACE", "0") == "1"
    if trace:
        _install_ntff_shim()

    input = np.asarray(input, np.float32)
    hx = np.asarray(hx, np.float32)
    ts = np.asarray(ts, np.float32)
    w1t, w2t, wht, w8g, biases, gate_bias = _prep_shared(
        np.asarray(Wb1, np.float32), np.asarray(bb1, np.float32),
        np.asarray(Wb2, np.float32), np.asarray(bb2, np.float32),
        np.asarray(W_ff1, np.float32), np.asarray(b_ff1, np.float32),
        np.asarray(W_ff2, np.float32), np.asarray(b_ff2, np.float32),
        np.asarray(W_ta, np.float32), np.asarray(b_ta, np.float32),
        np.asarray(W_tb, np.float32), np.asarray(b_tb, np.float32),
    )

    key = ("nc", gate_bias)
    if key not in _cache:
        _cache[key] = _build(gate_bias)
    nc = _cache[key]

    in_maps = []
    for c in range(N_CORES):
        sl = slice(c * BL, (c + 1) * BL)
        x_c = np.concatenate([input[sl], hx[sl]], axis=1)  # [BL, K1]
        xt_c = x_c.T.astype(_F16)  # [K1, BL]
        # pack [x^T(b0) | Wb1^T] so the first DMA per k-tile carries
        # exactly what chunk-b0's matmuls need; x^T(b1) rides separately
        # in k-grouped layout, and the k0 head piece [w1 n0 | xt_b0]
        # rides first as a contiguous block.
        l1p_c = np.concatenate([xt_c[:, :512], w1t], axis=1)
        l1h_c = np.ascontiguousarray(
            np.concatenate([w1t[:128, :128], xt_c[:128, :512]], axis=1)
        )
        xtb1_c = _kgroup(np.ascontiguousarray(xt_c[:, 512:]))
        tsb_c = np.ascontiguousarray(
            np.broadcast_to(ts[sl].reshape(1, BL), (128, BL))
        ).astype(_F16)
        in_maps.append(
            {
                "l1h": l1h_c,
                "l1p": l1p_c,
                "xtb1": xtb1_c,
                "w2t": w2t,
                "wht": wht,
                "w8g": w8g,
                "biases": biases,
                "tsb": tsb_c,
            }
        )

    res = run_bass_kernel_spmd(nc, in_maps, list(range(N_CORES)), trace=trace)
    LAST_EXEC_TIME_NS = res.exec_time_ns

    full = np.empty((B, HID), np.float32)
    for c in range(N_CORES):
        full[c * BL : (c + 1) * BL] = res.results[c]["out"].T.astype(np.float32)
    return full



# revision 27
# speedup vs baseline: 1.0619x; 1.0619x over previous
"""CfC cell (dense MLP) on 8 Trainium2 NeuronCores — data-parallel over batch.

Math (per sample):
    x  = concat(input, hx)                       # [1024]
    x1 = 1.7159*tanh(0.666*(Wb1 x + bb1))        # backbone 1, [1024]
    x2 = 1.7159*tanh(0.666*(Wb2 x1 + bb2))       # backbone 2, [1024]
    ff1 = tanh(W_ff1 x2 + b_ff1); ff2 = tanh(W_ff2 x2 + b_ff2)
    t   = sigmoid((W_ta x2 + b_ta)*ts + W_tb x2 + b_tb)
    out = ff1 + t*(ff2 - ff1)                    # [512]

Device layout: all activations live transposed as [feature_partition,
batch_free]; weights are host-pre-transposed to [K, N] so every GEMM is a
straight lhsT.T @ rhs chain with no on-device transposes. The 1.7159
LeCun gains are folded into the *next* layer's weights, the 0.666 input
scales into the ACT instruction's free affine + pre-scaled biases, so
each layer is exactly matmul-accumulate -> one ACT op.

The gate path (t_a, t_b heads) runs in fp8e4m3 with DoubleRow matmuls
(2x PE throughput): since ts is per-sample, (W_ta x2)*ts = W_ta (x2*ts),
so both gate GEMMs accumulate into ONE PSUM bank
    P = s * (W_ta (x2*ts) + W_tb x2),  s = 2048 (weight fp8 scaling)
and the sigmoid reads P directly with scale ±1/s (sigmoid(-v) = 1-t).
Gate biases are zero for this model; a general bias path is compiled in
only when they are nonzero. fp8 error on the gate path is damped by the
sigmoid slope: measured end-to-end rel err ~1.5e-2 < 2e-2 budget.
The backbone and ff GEMM inputs are fp16 (fp32 PSUM accumulation);
the head epilogue runs in fp16 and the output DMA is fp16.

Batch 8192 is split 1024/core across the 8 cores; weights are replicated.
"""
import os
from contextlib import ExitStack

import numpy as np
import ml_dtypes

IN, HID, BB, B = 512, 512, 1024, 8192
N_CORES = 8
BL = B // N_CORES        # 1024 batch rows per core
K1 = IN + HID            # contraction dim of backbone layer 1 (== BB here)
KT = K1 // 128           # 8 k-tiles (also BB//128)
NB = BL // 512           # 2 batch chunks of 512 (PSUM bank = 512 fp32)
NT1 = BB // 128          # 8 output tiles for backbone layers
NTH = HID // 128         # 4 output tiles per head
LA, LBc = 1.7159, 0.666  # LeCun tanh gain / input scale
GS = 2048.0              # fp8 gate-weight scale (power of 2)

_F16 = np.float16
_F8 = ml_dtypes.float8_e4m3

_cache: dict = {}

# Set by each kernel() call when tracing is enabled (BASS_KERNEL_TRACE=1).
LAST_EXEC_TIME_NS = None


def _install_ntff_shim():
    """Recreate the missing ``antenv.axon_hooks`` so trace=True works."""
    import sys, types, ctypes, contextlib

    if "antenv.axon_hooks" in sys.modules:
        return
    so_path = "/opt/axon/libaxon_pjrt.so"
    try:
        lib = ctypes.CDLL(so_path)
    except OSError:
        return
    if not hasattr(lib, "axon_start_nrt_profile"):
        return
    lib.axon_start_nrt_profile.argtypes = [ctypes.POINTER(ctypes.c_int64), ctypes.c_size_t]
    lib.axon_start_nrt_profile.restype = ctypes.c_int64
    lib.axon_stop_nrt_profile.argtypes = [ctypes.c_char_p]
    lib.axon_stop_nrt_profile.restype = ctypes.c_int64

    @contextlib.contextmanager
    def _hook(output_dir, device_ids):
        import jax

        jax.devices()
        if device_ids:
            ids = (ctypes.c_int64 * len(device_ids))(*device_ids)
            rc = lib.axon_start_nrt_profile(ids, len(device_ids))
        else:
            rc = lib.axon_start_nrt_profile(None, 0)
        if rc != 0:
            raise RuntimeError(f"axon_start_nrt_profile rc={rc}")
        try:
            yield
        finally:
            n = lib.axon_stop_nrt_profile(str(output_dir).encode())
            if n < 0:
                raise RuntimeError(f"axon_stop_nrt_profile rc={n}")

    mod = types.ModuleType("antenv.axon_hooks")
    mod.get_axon_ntff_profile_hook = lambda: _hook

    def set_axon_ntff_profile_hook(h):
        mod.get_axon_ntff_profile_hook = lambda: h

    mod.set_axon_ntff_profile_hook = set_axon_ntff_profile_hook
    sys.modules["antenv.axon_hooks"] = mod
    import antenv

    antenv.axon_hooks = mod


def _build(gate_bias: bool):
    from concourse import bacc, tile, mybir

    f32 = mybir.dt.float32
    f16 = mybir.dt.float16
    f8 = mybir.dt.float8e4
    Tanh = mybir.ActivationFunctionType.Tanh
    Sigm = mybir.ActivationFunctionType.Sigmoid
    DR = mybir.MatmulPerfMode.DoubleRow

    nc = bacc.Bacc("TRN2", target_bir_lowering=False, debug=False, num_devices=N_CORES)

    # xt and w1t packed side by side so one DMA per k-tile delivers a
    # complete (xt_k, w1_k) operand pair; columns are [xt_b0 | w1 | xt_b1].
    l1p = nc.declare_dram_parameter("l1p", [K1, BL + BB], f16, isOutput=False)
    w2t = nc.declare_dram_parameter("w2t", [BB, BB], f16, isOutput=False)
    wht = nc.declare_dram_parameter("wht", [BB, 2 * HID], f16, isOutput=False)
    w8g = nc.declare_dram_parameter("w8g", [128, KT, 2 * HID], f8, isOutput=False)
    biases = nc.declare_dram_parameter("biases", [128, 32], f32, isOutput=False)
    tsb = nc.declare_dram_parameter("tsb", [128, BL], f16, isOutput=False)
    out = nc.declare_dram_parameter("out", [HID, BL], f16, isOutput=True)

    with tile.TileContext(nc) as tc, ExitStack() as ctx:
        sb = ctx.enter_context(tc.tile_pool(name="sb", bufs=1))
        tmp = ctx.enter_context(tc.tile_pool(name="tmp", bufs=2))
        ps = ctx.enter_context(tc.tile_pool(name="ps", bufs=8, space="PSUM"))

        # The sync HWDGE ring carries the big weight stream in consumption
        # order; the small biases/tsb/w8g transfers ride the otherwise-idle
        # scalar HWDGE ring so they arrive early without delaying l1p k0.
        # l1p columns are [xt_b0 | w1 | xt_b1]; the first DMA per k-tile is
        # just the chunk-b0 matmuls need, so they start sooner and stay
        # PE-bound even when the DMA stream ramps slowly.
        # Input stream, in consumption order, serialized on the sync ring so
        # later transfers can't steal queue slots from the critical early
        # k-tiles. The k0 head piece [xt_b0 | w1 n0] is split off so the very
        # first matmul's operands land ~1us sooner; no PE warm-up — the first
        # real L1 matmuls ramp the clock while doing useful work.
        l1_t = [sb.tile([128, BL + BB], f16, tag=f"l1{k}", name=f"l1{k}") for k in range(KT)]
        nc.sync.dma_start(l1_t[0][:, :640], l1p[0:128, :640])
        nc.sync.dma_start(l1_t[0][:, 640:1024], l1p[0:128, 640:1024])
        nc.sync.dma_start(l1_t[0][:, 1024 : 512 + BB], l1p[0:128, 1024 : 512 + BB])
        nc.sync.dma_start(l1_t[1][:, :1024], l1p[128:256, :1024])
        nc.sync.dma_start(l1_t[1][:, 1024 : 512 + BB], l1p[128:256, 1024 : 512 + BB])
        nc.sync.dma_start(l1_t[2][:, :1024], l1p[256:384, :1024])
        nc.sync.dma_start(l1_t[2][:, 1024 : 512 + BB], l1p[256:384, 1024 : 512 + BB])
        for k in range(3, KT):
            rows = slice(128 * k, 128 * (k + 1))
            nc.sync.dma_start(l1_t[k][:, : 512 + BB], l1p[rows, : 512 + BB])
        for k in range(KT):
            rows = slice(128 * k, 128 * (k + 1))
            nc.sync.dma_start(l1_t[k][:, 512 + BB :], l1p[rows, 512 + BB :])
        bias_t = sb.tile([128, 32], f32, tag="bias")
        nc.scalar.dma_start(bias_t[:], biases[:])
        w2_t = [sb.tile([128, BB], f16, tag=f"w2{k}", name=f"w2{k}") for k in range(KT)]
        wh_t = [sb.tile([128, 2 * HID], f16, tag=f"wh{k}", name=f"wh{k}") for k in range(KT)]
        for k in range(KT):
            nc.sync.dma_start(w2_t[k][:], w2t[128 * k : 128 * (k + 1), :])
        tsb_t = sb.tile([128, BL], f16, tag="tsb")
        nc.sync.dma_start(tsb_t[:], tsb[:])
        w8g_t = sb.tile([128, KT, 2 * HID], f8, tag="w8g")
        nc.sync.dma_start(w8g_t[:], w8g[:])
        for k in range(KT):
            nc.sync.dma_start(wh_t[k][:], wht[128 * k : 128 * (k + 1), :])

        # HAM warm-up: dummy matmuls on a zeroed tile ramp the PE clock while
        # the first real operands stream in (~10.9us to first l1p piece), so
        # the real matmul stream starts at full speed.
        warm = sb.tile([128, 512], f16, tag="warm")
        nc.vector.memset(warm[:], 0.0)
        wacc = ps.tile([128, 512], f32, tag="ps", name="warm_ps")
        N_WARM = 7
        for i in range(N_WARM):
            nc.tensor.matmul(
                wacc[:], warm[:, :128], warm[:], start=(i == 0), stop=(i == N_WARM - 1)
            )

        h1_t = [sb.tile([128, BL], f16, tag=f"h1{n}", name=f"h1{n}") for n in range(NT1)]
        h2_t = [sb.tile([128, BL], f16, tag=f"h2{n}", name=f"h2{n}") for n in range(NT1)]
        # fp8 copies of x2 (and x2*ts) for the DoubleRow gate GEMMs, laid
        # out [128, kgroup, batch] so dim1 pairs feed DoubleRow directly.
        x28_t = sb.tile([128, KT, BL], f8, tag="x28")
        x2s8_t = sb.tile([128, KT, BL], f8, tag="x2s8")

        # backbone layer 1: h1 = tanh(0.666*(W1 x) + 0.666*bb1), fp16 out.
        # k-OUTER so each (xt_k, w1_k) pair is consumed for all 8 n-tiles the
        # moment its DMA lands — the PE streams during the input transfer
        # instead of stalling on the last k-tile. Needs 8 live PSUM banks.
        for b in range(NB):
            bsl = slice(512 * b, 512 * (b + 1))
            accs1 = [
                ps.tile([128, 512], f32, tag="ps", name=f"ps1_{b}_{n}")
                for n in range(NT1)
            ]
            # column layout of l1_t: [xt_b0 (512) | w1 (1024) | xt_b1 (512)]
            rhs_lo = 0 if b == 0 else 512 + BB
            for k in range(KT):
                for n in range(NT1):
                    nc.tensor.matmul(
                        accs1[n][:],
                        l1_t[k][:, 512 + 128 * n : 512 + 128 * (n + 1)],
                        l1_t[k][:, rhs_lo : rhs_lo + 512],
                        start=(k == 0),
                        stop=(k == KT - 1),
                    )
            for n in range(NT1):
                nc.scalar.activation(
                    h1_t[n][:, bsl], accs1[n][:], Tanh, bias=bias_t[:, n : n + 1], scale=LBc
                )

        # backbone layer 2 (1.7159 folded into w2t on host). Each n-tile's
        # PSUM is evicted to fp16 h2 (scalar ACT); the fp8 gate operands are
        # produced on the vector engine from the fp16 copy.
        for b in range(NB):
            bsl = slice(512 * b, 512 * (b + 1))
            for n in range(NT1):
                acc = ps.tile([128, 512], f32, tag="ps", name=f"ps2_{b}_{n}")
                for k in range(KT):
                    nc.tensor.matmul(
                        acc[:],
                        w2_t[k][:, 128 * n : 128 * (n + 1)],
                        h1_t[k][:, bsl],
                        start=(k == 0),
                        stop=(k == KT - 1),
                    )
                nc.scalar.activation(
                    h2_t[n][:, bsl], acc[:], Tanh, bias=bias_t[:, 8 + n : 9 + n], scale=LBc
                )
                nc.vector.tensor_scalar_add(x28_t[:, n, bsl], h2_t[n][:, bsl], 0.0)
                nc.vector.tensor_mul(x2s8_t[:, n, bsl], h2_t[n][:, bsl], tsb_t[:, bsl])

        if gate_bias:
            # general path: gbias_n = GS*(b_ta*ts + b_tb), added to the gate
            # PSUM before the sigmoids (cols 24..27 = GS*b_ta, 28..31 = GS*b_tb)
            gb_t = [sb.tile([128, BL], f32, tag=f"gb{n}", name=f"gb{n}") for n in range(NTH)]
            for n in range(NTH):
                nc.vector.tensor_scalar(
                    gb_t[n][:], tsb_t[:], bias_t[:, 24 + n : 25 + n],
                    bias_t[:, 28 + n : 29 + n],
                    op0=mybir.AluOpType.mult, op1=mybir.AluOpType.add,
                )

        # heads: the fused gate PSUM (t_a*ts + t_b, fp8 DoubleRow) comes
        # first so the sigmoid chain overlaps the ff GEMMs; only
        # ACT(ff2) -> mul -> add trails the last matmul.
        def head_group(b, n, lo, w, sub, ring):
            """One head pipeline over batch cols [512*b+lo, 512*b+lo+w)."""
            bsl = slice(512 * b + lo, 512 * b + lo + w)
            sfx = f"_{b}_{n}_{sub}"

            gate_ps = ps.tile([128, w], f32, tag="ps", name="ps_g" + sfx)
            for h in range(2):  # 0: ta on x2*ts, 1: tb on x2
                rhs_t = x2s8_t if h == 0 else x28_t
                col = 512 * h + 128 * n
                for i in range(KT // 2):
                    nc.tensor.matmul(
                        gate_ps[:],
                        w8g_t[:, 2 * i : 2 * i + 2, col : col + 128],
                        rhs_t[:, 2 * i : 2 * i + 2, bsl],
                        start=(h == 0 and i == 0),
                        stop=(h == 1 and i == KT // 2 - 1),
                        perf_mode=DR,
                    )
            if gate_bias:
                nc.vector.tensor_add(gate_ps[:], gate_ps[:], gb_t[n][:, bsl])
            tpos = tmp.tile([128, w], f16, tag="tpos", name="tpos" + sfx)
            nc.scalar.activation(tpos[:], gate_ps[:], Sigm, scale=1.0 / GS)
            tneg = tmp.tile([128, w], f16, tag="tneg", name="tneg" + sfx)
            nc.scalar.activation(tneg[:], gate_ps[:], Sigm, scale=-1.0 / GS)

            def ff_mms(h, acc):
                col = 512 * h + 128 * n
                for k in range(KT):
                    nc.tensor.matmul(
                        acc[:],
                        wh_t[k][:, col : col + 128],
                        h2_t[k][:, bsl],
                        start=(k == 0),
                        stop=(k == KT - 1),
                    )

            def bc(h):
                c = 16 + 4 * h + n
                return bias_t[:, c : c + 1]

            ff1_ps = ps.tile([128, w], f32, tag="ps", name="ps_ff1" + sfx)
            ff_mms(0, ff1_ps)
            ff1 = tmp.tile([128, w], f16, tag="ff1", name="ff1" + sfx)
            nc.scalar.activation(ff1[:], ff1_ps[:], Tanh, bias=bc(0))
            u = tmp.tile([128, w], f16, tag="u", name="u" + sfx)
            nc.vector.tensor_mul(u[:], tneg[:], ff1[:])  # (1-t)*ff1

            ff2_ps = ps.tile([128, w], f32, tag="ps", name="ps_ff2" + sfx)
            ff_mms(1, ff2_ps)
            ff2 = tmp.tile([128, w], f16, tag="ff2", name="ff2" + sfx)
            nc.scalar.activation(ff2[:], ff2_ps[:], Tanh, bias=bc(1))
            m2 = tmp.tile([128, w], f16, tag="m2", name="m2" + sfx)
            nc.vector.tensor_mul(m2[:], tpos[:], ff2[:])  # t*ff2
            o = tmp.tile([128, w], f16, tag="o", name="o" + sfx)
            nc.vector.tensor_add(o[:], u[:], m2[:])
            ring.dma_start(out[128 * n : 128 * (n + 1), bsl], o[:])

        for b in range(NB):
            for n in range(NTH):
                if b == NB - 1 and n == NTH - 1:
                    # Final group: two half-batch pipelines, so the first
                    # half's epilogue overlaps the second half's GEMMs and
                    # only a 256-wide ACT->mul->add->DMA trails the last MM.
                    # Their DMAs ride the idle scalar ring so the flush isn't
                    # stuck behind the sync ring's end-of-stream work.
                    head_group(b, n, 0, 256, 0, nc.scalar)
                    head_group(b, n, 256, 256, 1, nc.scalar)
                else:
                    head_group(b, n, 0, 512, 0, nc.sync)

    nc.finalize()
    return nc


def _prep_shared(Wb1, bb1, Wb2, bb2, W_ff1, b_ff1, W_ff2, b_ff2, W_ta, b_ta, W_tb, b_tb):
    """Host-side weight layout: transpose to [K, N], fold LeCun gains."""
    w1t = Wb1.T.astype(_F16)
    w2t = np.ascontiguousarray((LA * Wb2).T).astype(_F16)
    wht = np.ascontiguousarray(
        np.concatenate([(LA * W).T for W in (W_ff1, W_ff2)], axis=1)
    ).astype(_F16)
    # gate weights: fp8 e4m3, scaled by GS, DoubleRow layout [128, kgroup, M]
    wg = np.concatenate([(GS * LA * W).T for W in (W_ta, W_tb)], axis=1)  # [K, 1024]
    w8g = np.ascontiguousarray(
        wg.reshape(KT, 128, 2 * HID).transpose(1, 0, 2)
    ).astype(_F8)

    biases = np.zeros((128, 32), np.float32)
    biases[:, 0:8] = (LBc * bb1).reshape(8, 128).T
    biases[:, 8:16] = (LBc * bb2).reshape(8, 128).T
    for h, bh in enumerate((b_ff1, b_ff2)):
        biases[:, 16 + 4 * h : 20 + 4 * h] = bh.reshape(4, 128).T
    gate_bias = bool(np.any(b_ta) or np.any(b_tb))
    if gate_bias:
        biases[:, 24:28] = (GS * b_ta).reshape(4, 128).T
        biases[:, 28:32] = (GS * b_tb).reshape(4, 128).T
    return w1t, w2t, wht, w8g, biases, gate_bias


def kernel(input, hx, ts, Wb1, bb1, Wb2, bb2, W_ff1, b_ff1, W_ff2, b_ff2, W_ta, b_ta, W_tb, b_tb):
    global LAST_EXEC_TIME_NS
    from concourse.bass_utils import run_bass_kernel_spmd

    trace = os.environ.get("BASS_KERNEL_TRACE", "0") == "1"
    if trace:
        _install_ntff_shim()

    input = np.asarray(input, np.float32)
    hx = np.asarray(hx, np.float32)
    ts = np.asarray(ts, np.float32)
    w1t, w2t, wht, w8g, biases, gate_bias = _prep_shared(
        np.asarray(Wb1, np.float32), np.asarray(bb1, np.float32),
        np.asarray(Wb2, np.float32), np.asarray(bb2, np.float32),
        np.asarray(W_ff1, np.float32), np.asarray(b_ff1, np.float32),
        np.asarray(W_ff2, np.float32), np.asarray(b_ff2, np.float32),
        np.asarray(W_ta, np.float32), np.asarray(b_ta, np.float32),
        np.asarray(W_tb, np.float32), np.asarray(b_tb, np.float32),
    )

    key = ("nc", gate_bias)
    if key not in _cache:
        _cache[key] = _build(gate_bias)
    nc = _cache[key]

    in_maps = []
    for c in range(N_CORES):
        sl = slice(c * BL, (c + 1) * BL)
        x_c = np.concatenate([input[sl], hx[sl]], axis=1)  # [BL, K1]
        xt_c = x_c.T.astype(_F16)  # [K1, BL]
        # pack [x^T(b0) | Wb1^T | x^T(b1)] so the first DMA per k-tile
        # carries exactly what chunk-b0's matmuls need
        l1p_c = np.concatenate([xt_c[:, :512], w1t, xt_c[:, 512:]], axis=1)
        tsb_c = np.ascontiguousarray(
            np.broadcast_to(ts[sl].reshape(1, BL), (128, BL))
        ).astype(_F16)
        in_maps.append(
            {
                "l1p": l1p_c,
                "w2t": w2t,
                "wht": wht,
                "w8g": w8g,
                "biases": biases,
                "tsb": tsb_c,
            }
        )

    res = run_bass_kernel_spmd(nc, in_maps, list(range(N_CORES)), trace=trace)
    LAST_EXEC_TIME_NS = res.exec_time_ns

    full = np.empty((B, HID), np.float32)
    for c in range(N_CORES):
        full[c * BL : (c + 1) * BL] = res.results[c]["out"].T.astype(np.float32)
    return full

